# revision 7
# baseline (speedup 1.0000x reference)
"""AttentionPool (single CLS query over ragged segments) on 8 TRN2 NeuronCores.

v3 design (DMA-bound; the original version was PE-bound on device-side
transposes):
  - Host folds the CLS query into the key projection:
        wq[i, h] = softmax_scale * sum_{j in head h} cls[j] * W_k[j, i]
    so scores[t, h] = sum_i embed[t, i] * wq[i, h].  Key bias and softmax
    max-subtraction shift scores by a per-(head, segment) constant that
    cancels in softmax => omitted (|s| <~ 30 << 88, exp stays finite in f32).
  - Host pushes embed in BOTH layouts so the device never transposes x:
      xt: d-major bf16        (feeds the scores matmul directly)
      xn: token-major fp8e3m4 (the num matmul moving operand; e3m4's 4
          mantissa bits keep the pooled-output error ~1.8% < 2% budget for
          the canonical 2048-token segments; bf16 stationary x fp8 moving
          matmuls verified bit-exact on HW.  If any segment is shorter than
          1536 tokens the pooling averages fewer values and fp8 noise would
          grow, so the host falls back to bf16 xn automatically.)
    Both tensors are pre-tiled on the host into the exact SBUF layout so each
    DMA line is one contiguous multi-KB run per partition.
  - Device per segment: scores = wq.T @ xt tiles (PE, bf16), exp on ACT with
    per-quarter denominator accumulation, p transposed token-major via DVE
    32x32 stream transposes (PE untouched), num[h,:] += p_chunk.T @ xn_chunk
    accumulated in PSUM over the segment.  Scores of quarter i+1 are emitted
    before the num matmuls of quarter i so the PE never waits on the
    ACT-exp / DVE-transpose chain.
  - Device outputs raw num [H, D] and per-quarter denoms; the host does the
    final out[i] = num[head(i), i] / denom[head(i)] (trivial numpy).

Self-contained: hardcodes the problem shapes; handles arbitrary cu_lens by
padding each segment slot to a fixed chunk grid (masked), which degenerates
to zero overhead for the expected equal-length segmentation.
"""

import math

import numpy as np

H = 20        # heads
D = 1280      # embed dim
DH = D // H   # head dim (64)
P = 128       # partitions
DC = D // P   # 10 d-chunks
NCORES = 8
QCH = 4       # chunks per quarter (512 tokens)


def _ceil_div(a, b):
    return -(-a // b)


def _halves(K):
    """Token-halved loads when quarters stay within a half."""
    return 2 if (K % 2 == 0 and (K // 2) % QCH == 0) else 1


def _build_program(S, K, use_mask, xn_lowp):
    """SPMD Bass program: S segment slots x K chunks x 128 tokens per core."""
    import concourse.tile as tile
    from concourse import bacc, mybir

    f32 = mybir.dt.float32
    bf16 = mybir.dt.bfloat16
    xn_dt = mybir.dt.float8e3 if xn_lowp else bf16
    Exp = mybir.ActivationFunctionType.Exp

    L = K * P                      # padded tokens per slot
    NQ = _ceil_div(K, QCH)         # quarters per slot
    HV = _halves(K)
    KH = K // HV                   # chunks per half
    LH = KH * P                    # tokens per half

    nc = bacc.Bacc()
    # pre-tiled on host: [slot, half, partition, flat SBUF bytes]
    xt = nc.dram_tensor("xt", [S, HV, P, DC * LH], bf16, kind="ExternalInput")
    xn = nc.dram_tensor("xn", [S, HV, P, KH * D], xn_dt, kind="ExternalInput")
    wqd = nc.dram_tensor("wqd", [D, H], bf16, kind="ExternalInput")
    maskin = None
    if use_mask:
        maskin = nc.dram_tensor("maskin", [S * L], f32, kind="ExternalInput")
    onum = nc.dram_tensor("onum", [S * H, D], f32, kind="ExternalOutput")
    oden = nc.dram_tensor("oden", [S * H, NQ], f32, kind="ExternalOutput")

    with tile.TileContext(nc) as tc:
        with tc.tile_pool(name="persist", bufs=1) as persist:
            wq_sb = persist.tile([P, DC, H], bf16)
            nc.sync.dma_start(
                out=wq_sb, in_=wqd[:, :].rearrange("(dc p) h -> p dc h", p=P))

            with tc.tile_pool(name="xt", bufs=2) as xt_pool, \
                 tc.tile_pool(name="xn", bufs=2) as xn_pool, \
                 tc.tile_pool(name="pp", bufs=2) as pp_pool, \
                 tc.tile_pool(name="pt", bufs=2) as pt_pool, \
                 tc.tile_pool(name="small", bufs=2) as small_pool, \
                 tc.tile_pool(name="ps_s", bufs=2, space="PSUM") as ps_s_pool, \
                 tc.tile_pool(name="ps_n", bufs=2, space="PSUM") as ps_n_pool:

                state = {}   # seg -> (xt_t, xn_t, p_sb, pt_t, dens)
                pnums = {}   # seg -> psum num tile

                def emit_seg_loads(seg):
                    xt_t = xt_pool.tile([P, HV, DC, LH], bf16, tag="xt")
                    xn_t = xn_pool.tile([P, HV, KH, D], xn_dt, tag="xn")
                    for hh in range(HV):
                        nc.sync.dma_start(
                            out=xt_t[:, hh, :, :], in_=xt[seg, hh, :, :])
                        nc.scalar.dma_start(
                            out=xn_t[:, hh, :, :], in_=xn[seg, hh, :, :])
                    p_sb = pp_pool.tile([32, L], bf16, tag="p")
                    pt_t = pt_pool.tile([P, K, 32], bf16, tag="pt")
                    dens = small_pool.tile([32, NQ], f32, tag="dens")
                    state[seg] = (xt_t, xn_t, p_sb, pt_t, dens)

                def emit_scores(seg, q):
                    (xt_t, xn_t, p_sb, pt_t, dens) = state[seg]
                    qc = min(QCH, K - q * QCH)
                    cols = qc * P
                    off = q * QCH * P          # token offset within slot
                    hh = off // LH
                    offh = off - hh * LH
                    sc = ps_s_pool.tile([H, QCH * P], f32, tag="sc")
                    for dc in range(DC):
                        nc.tensor.matmul(
                            sc[:, :cols],
                            lhsT=wq_sb[:, dc, :],
                            rhs=xt_t[:, hh, dc, offh:offh + cols],
                            start=(dc == 0), stop=(dc == DC - 1))
                    # exp (h-major) + denominator
                    if use_mask:
                        nc.scalar.activation(
                            out=p_sb[0:H, off:off + cols], in_=sc[:, :cols],
                            func=Exp)
                        msk = small_pool.tile([H, QCH * P], f32, tag="msk")
                        nc.gpsimd.dma_start(
                            out=msk[:, :cols],
                            in_=maskin[seg * L + off:seg * L + off + cols]
                            .partition_broadcast(H))
                        nc.vector.tensor_mul(
                            p_sb[0:H, off:off + cols],
                            p_sb[0:H, off:off + cols], msk[:, :cols])
                        nc.vector.tensor_reduce(
                            out=dens[0:H, q:q + 1],
                            in_=p_sb[0:H, off:off + cols],
                            axis=mybir.AxisListType.X, op=mybir.AluOpType.add)
                    else:
                        nc.scalar.activation(
                            out=p_sb[0:H, off:off + cols], in_=sc[:, :cols],
                            func=Exp, accum_out=dens[0:H, q:q + 1])
                    # token-major p via DVE 32x32 stream transposes
                    for c in range(q * QCH, q * QCH + qc):
                        for j in range(4):
                            t0 = c * P + 32 * j
                            nc.vector.transpose(
                                out=pt_t[32 * j:32 * j + 32, c, :],
                                in_=p_sb[:, t0:t0 + 32])
                    return (seg, q, qc)

                def emit_num(job):
                    (seg, q, qc) = job
                    (xt_t, xn_t, p_sb, pt_t, dens) = state[seg]
                    if q == 0:
                        pnums[seg] = ps_n_pool.tile(
                            [H, D], f32, tag="pnum", name="pnum")
                    pnum = pnums[seg]
                    for c in range(q * QCH, q * QCH + qc):
                        hh = c // KH
                        ch = c - hh * KH
                        for n0, n1 in ((0, 512), (512, 1024), (1024, D)):
                            nc.tensor.matmul(
                                pnum[:, n0:n1],
                                lhsT=pt_t[:, c, 0:H],
                                rhs=xn_t[:, hh, ch, n0:n1],
                                start=(c == 0), stop=(c == K - 1),
                                skip_group_check=True)
                    if q == NQ - 1:
                        onum_sb = small_pool.tile([H, D], f32, tag="onum")
                        nc.vector.tensor_copy(out=onum_sb, in_=pnum)
                        nc.gpsimd.dma_start(
                            out=onum[seg * H:(seg + 1) * H, :], in_=onum_sb)
                        nc.gpsimd.dma_start(
                            out=oden[seg * H:(seg + 1) * H, :],
                            in_=dens[0:H, :])
                        del pnums[seg]
                        del state[seg]

                jobs = [(seg, q) for seg in range(S) for q in range(NQ)]
                pending = None
                emit_seg_loads(0)
                for (seg, q) in jobs:
                    if q == 0 and seg + 1 < S:
                        emit_seg_loads(seg + 1)
                    job = emit_scores(seg, q)
                    if pending is not None:
                        emit_num(pending)
                    pending = job
                emit_num(pending)
    nc.finalize()
    return nc


def _plan(cu_lens):
    """Host-side sharding plan. assignments[core] = [(slot, seg, start, end)]."""
    cu = [int(v) for v in cu_lens]
    n = len(cu) - 1
    lens = [cu[i + 1] - cu[i] for i in range(n)]
    S = _ceil_div(n, NCORES)
    max_len = max(lens) if lens else 1
    K = max(1, _ceil_div(max_len, P))
    use_mask = (n != S * NCORES) or any(l != K * P for l in lens)
    # fp8e3m4 values are safe when every segment pools >= ~1536 tokens
    xn_lowp = bool(lens) and min(lens) >= 1536
    assignments = []
    for i in range(NCORES):
        rows = []
        for s in range(S):
            seg = i * S + s
            if seg < n:
                rows.append((s, seg, cu[seg], cu[seg + 1]))
        assignments.append(rows)
    return S, K, assignments, use_mask, xn_lowp


def _tile_host(block_t, block_n, K, HV):
    """Pre-tile one slot into the device SBUF layouts.

    block_t: [D, L] (d-major), block_n: [L, D] (token-major).
    Returns xt_slot [HV, P, DC*LH], xn_slot [HV, P, KH*D].
    """
    L = K * P
    KH = K // HV
    LH = KH * P
    # xt: [D, L] -> [DC, P, HV, LH] -> [HV, P, DC, LH]
    xt_s = block_t.reshape(DC, P, HV, LH).transpose(2, 1, 0, 3)
    # xn: [L, D] -> [HV, KH, P, D] -> [HV, P, KH, D]
    xn_s = block_n.reshape(HV, KH, P, D).transpose(0, 2, 1, 3)
    return (xt_s.reshape(HV, P, DC * LH), xn_s.reshape(HV, P, KH * D))


def prepare(cls, embed, cu_lens, W_k):
    """Host-side: fold wq, build both embed layouts per core, build program."""
    import ml_dtypes
    bf16 = ml_dtypes.bfloat16
    f83 = ml_dtypes.float8_e3m4

    cls = np.asarray(cls, dtype=np.float64).reshape(D)
    embed = np.asarray(embed, dtype=np.float32)
    W_k = np.asarray(W_k, dtype=np.float64)
    cu = np.asarray(cu_lens).astype(np.int64)
    n = cu.shape[0] - 1

    S, K, assignments, use_mask, xn_lowp = _plan(cu)
    L = K * P
    HV = _halves(K)
    KH = K // HV
    nc = _build_program(S, K, use_mask, xn_lowp)
    xn_np_dt = f83 if xn_lowp else bf16

    # wq[i, h] = scale * sum_{j in head h} cls[j] W_k[j, i]
    scale = 1.0 / math.sqrt(DH)
    wq = np.einsum("hj,hji->ih", cls.reshape(H, DH),
                   W_k.reshape(H, DH, D)) * scale
    wq_bf = wq.astype(np.float32).astype(bf16)

    emb_bf = embed.astype(bf16)
    emb_lp = embed.astype(xn_np_dt)

    in_maps = []
    for i in range(NCORES):
        rows = assignments[i]
        xt_np = np.zeros((S, HV, P, DC * KH * P), dtype=bf16)
        xn_np = np.zeros((S, HV, P, KH * D), dtype=xn_np_dt)
        mask = np.zeros((S * L,), dtype=np.float32) if use_mask else None
        for (s, _seg, start, end) in rows:
            ln = end - start
            bt = np.zeros((D, L), dtype=bf16)
            bn = np.zeros((L, D), dtype=xn_np_dt)
            bt[:, :ln] = emb_bf[start:end].T
            bn[:ln] = emb_lp[start:end]
            xt_np[s], xn_np[s] = _tile_host(bt, bn, K, HV)
            if use_mask:
                mask[s * L:s * L + ln] = 1.0
        m = {"xt": xt_np, "xn": xn_np, "wqd": wq_bf}
        if use_mask:
            m["maskin"] = mask
        in_maps.append(m)
    return nc, in_maps, assignments, n


def gather(results, assignments, n):
    head = np.arange(D) // DH
    full = np.zeros((n, 1, D), dtype=np.float32)
    for i in range(NCORES):
        onum = np.asarray(results[i]["onum"])      # (S*H, D)
        oden = np.asarray(results[i]["oden"])      # (S*H, NQ)
        for (s, seg, _start, _end) in assignments[i]:
            num = onum[s * H:(s + 1) * H, :]
            den = oden[s * H:(s + 1) * H, :].sum(axis=1)
            full[seg, 0, :] = num[head, np.arange(D)] / den[head]
    return full


def kernel(cls, embed, cu_lens, max_len, W_k, b_k):
    from concourse.bass_utils import run_bass_kernel_spmd

    nc, in_maps, assignments, n = prepare(cls, embed, cu_lens, W_k)
    res = run_bass_kernel_spmd(nc, in_maps, core_ids=list(range(NCORES)))
    return gather(res.results, assignments, n)
